# revision 9
# baseline (speedup 1.0000x reference)
"""CharRNN (2-layer masked LSTM + MLP head) Trainium2 Bass kernel.

Strategy: pure data parallel over batch (512 seqs -> 8 cores x 64).
Per core, a staggered 2-layer LSTM scan: superstep s computes layer-1
step s and layer-2 step s-1 side by side in 128-wide tiles
(64 batch cols x 2 layers).  Key transformations:

- Masked-LSTM semantics: because the mask (t < len) is monotone and the
  final output is zeroed where mask==0, running the scan UNMASKED gives
  bit-identical results in the valid region; masking reduces to zeroing
  the final logits (done host-side after gather).
- Embedding lookup + input projection fused: one matmul against a
  per-chunk one-hot matrix, with lhsT = emb @ w_ih1.T precomputed on
  host (tiny).  One-hot built on device: K=1 matmul broadcasts the
  token row across 100 partitions, then tensor_scalar(is_equal) against
  an iota column.
- Single sigmoid for all 4 gates: tanh(x) = 2*sigmoid(2x) - 1, with the
  x2 folded into the g-gate rows of all weight matrices.  State is kept
  scaled: h_hat = h/2, c_hat = c/2, with the compensating x2 folded into
  every matmul that consumes h (w_hh1, w_ih2, w_hh2, lw1).
- Cell update in 3 DVE ops (scalar_tensor_tensor fusions):
    r2 = (p - 0.5) * s_i          (= s_i*tanh(g)/2)
    fc = s_f * c_hat_prev
    c_hat = fc + r2
    u  = sigmoid(4*c_hat)          (ACT, scale=4 is free)
    h_hat = (u - 0.5) * s_o        (= h/2)
- Biases enter via K=1 rank-1 matmuls (bias_row (x) ones_row).
- Gate rows live in 32-aligned blocks (i:0-29, f:32-61, o:64-93,
  g:96-125) so every engine slice has a legal start partition.
- MLP head + output DMA pipelined per 8 supersteps, overlapped with the
  scan.  Device emits logits as [vocab, T*64]; host transposes/masks.
"""

import os
import sys

import numpy as np

sys.path.insert(0, "/opt/trn_rl_repo")

VOCAB, EMBED, HID, B, T = 100, 50, 30, 512, 512
NCORES = 8
BS = B // NCORES            # 64 sequences per core
CH = 64                     # supersteps per chunk
G4H = 128                   # padded gate dim (4 blocks of 32)

_PROGRAM = None             # (nc, input_names) cache


def _gate_perm():
    """Map torch gate-row order [i,f,g,o] (each 30) to 32-aligned blocks
    i->0:30, f->32:62, o->64:94, g->96:126 inside a 128-row layout."""
    perm = np.zeros(4 * HID, dtype=np.int64)
    perm[0:30] = np.arange(30)            # i
    perm[30:60] = 32 + np.arange(30)      # f
    perm[60:90] = 96 + np.arange(30)      # g
    perm[90:120] = 64 + np.arange(30)     # o
    return perm


def _expand_gate_cols(w):
    """w: [..., 120] gate-last -> [..., 128] permuted, g-cols doubled."""
    perm = _gate_perm()
    out = np.zeros(w.shape[:-1] + (G4H,), np.float32)
    out[..., perm] = w
    out[..., 96:126] *= 2.0
    return out


def build_program(t_steps=T):
    import concourse.bass as bass
    import concourse.bacc as bacc
    import concourse.mybir as mybir
    import concourse.tile as tile

    f32 = mybir.dt.float32
    A = mybir.AluOpType
    AF = mybir.ActivationFunctionType

    n_chunks = (t_steps + CH - 1) // CH
    ncols = t_steps * BS

    nc = bacc.Bacc()

    tok = nc.declare_dram_parameter("tok", [1, ncols], f32, isOutput=False)
    mx = nc.declare_dram_parameter("mx", [VOCAB, G4H], f32, isOutput=False)
    w1h = nc.declare_dram_parameter("w1h", [HID, G4H], f32, isOutput=False)
    w2ih = nc.declare_dram_parameter("w2ih", [HID, G4H], f32, isOutput=False)
    w2hh = nc.declare_dram_parameter("w2hh", [HID, G4H], f32, isOutput=False)
    b1 = nc.declare_dram_parameter("b1", [1, G4H], f32, isOutput=False)
    b2 = nc.declare_dram_parameter("b2", [1, G4H], f32, isOutput=False)
    hw1 = nc.declare_dram_parameter("hw1", [HID, HID], f32, isOutput=False)
    hw2 = nc.declare_dram_parameter("hw2", [HID, VOCAB], f32, isOutput=False)
    lb1 = nc.declare_dram_parameter("lb1", [HID, 1], f32, isOutput=False)
    lb2 = nc.declare_dram_parameter("lb2", [VOCAB, 1], f32, isOutput=False)
    iota = nc.declare_dram_parameter("iota", [VOCAB, 1], f32, isOutput=False)
    ones = nc.declare_dram_parameter("ones", [1, 512], f32, isOutput=False)
    out = nc.declare_dram_parameter("out", [VOCAB, ncols], f32, isOutput=True)

    with tile.TileContext(nc) as tc:
        from contextlib import ExitStack
        with ExitStack() as ctx:
            cpool = ctx.enter_context(tc.tile_pool(name="consts", bufs=1))
            ohpool = ctx.enter_context(tc.tile_pool(name="oh", bufs=2))
            tkpool = ctx.enter_context(tc.tile_pool(name="tk", bufs=2))
            tbps = ctx.enter_context(
                tc.tile_pool(name="tbps", bufs=1, space="PSUM"))
            gps = ctx.enter_context(
                tc.tile_pool(name="gps", bufs=3, space="PSUM"))
            sps = ctx.enter_context(
                tc.tile_pool(name="sps", bufs=2, space="PSUM"))
            spool = ctx.enter_context(tc.tile_pool(name="sg", bufs=3))
            cc = ctx.enter_context(tc.tile_pool(name="cc", bufs=3))
            r2p = ctx.enter_context(tc.tile_pool(name="r2", bufs=3))
            fcp = ctx.enter_context(tc.tile_pool(name="fcb", bufs=3))
            up = ctx.enter_context(tc.tile_pool(name="ub", bufs=3))
            seqp = ctx.enter_context(tc.tile_pool(name="seq", bufs=2))
            hmps = ctx.enter_context(
                tc.tile_pool(name="hmps", bufs=1, space="PSUM"))
            hmp = ctx.enter_context(tc.tile_pool(name="hm", bufs=2))
            lgps = ctx.enter_context(
                tc.tile_pool(name="lgps", bufs=1, space="PSUM"))
            lgp = ctx.enter_context(tc.tile_pool(name="lg", bufs=3))

            def load_const(ap, shape, tag):
                t = cpool.tile(list(shape), f32, tag=tag)
                nc.sync.dma_start(out=t[:], in_=ap[:])
                return t

            mx_t = load_const(mx, (VOCAB, G4H), "mx")
            w1h_t = load_const(w1h, (HID, G4H), "w1h")
            w2ih_t = load_const(w2ih, (HID, G4H), "w2ih")
            w2hh_t = load_const(w2hh, (HID, G4H), "w2hh")
            b1_t = load_const(b1, (1, G4H), "b1")
            b2_t = load_const(b2, (1, G4H), "b2")
            hw1_t = load_const(hw1, (HID, HID), "hw1")
            hw2_t = load_const(hw2, (HID, VOCAB), "hw2")
            lb1_t = load_const(lb1, (HID, 1), "lb1")
            lb2_t = load_const(lb2, (VOCAB, 1), "lb2")
            iota_t = load_const(iota, (VOCAB, 1), "iota")
            ones_t = load_const(ones, (1, 512), "ones")

            c_prev = None           # AP of previous c_hat tile
            seq_cur = None          # current chunk h-seq tile
            seq_prev_tile = None    # previous chunk h-seq tile
            oh_cur = None

            def head_group(seq_tile, blk0, nblk, t0):
                """MLP head over ĥ2 blocks [blk0, blk0+nblk) of seq_tile;
                t0 = timestep of block blk0; writes out cols t0*BS.."""
                n = nblk * BS
                r = seq_tile[0:HID, :].rearrange(
                    "p (j c) -> p j c", c=2 * BS)
                rhs = r[:, blk0:blk0 + nblk, BS:2 * BS]
                hm_ps = hmps.tile([HID, 512], f32)
                nc.tensor.matmul(hm_ps[:, 0:n], hw1_t[:], rhs,
                                 start=True, stop=True)
                hm = hmp.tile([32, 512], f32)
                nc.scalar.activation(hm[0:HID, 0:n], hm_ps[0:HID, 0:n],
                                     AF.Relu, bias=lb1_t[:, 0:1])
                lg_ps = lgps.tile([VOCAB, 512], f32)
                nc.tensor.matmul(lg_ps[:, 0:n], hw2_t[:], hm[0:HID, 0:n],
                                 start=True, stop=True)
                lg = lgp.tile([VOCAB, 512], f32)
                nc.scalar.activation(lg[:, 0:n], lg_ps[:, 0:n],
                                     AF.Identity, bias=lb2_t[:, 0:1])
                nc.sync.dma_start(out=out[:, t0 * BS:t0 * BS + n],
                                  in_=lg[:, 0:n])

            for s in range(t_steps + 1):
                chunk, j = s // CH, s % CH
                has1 = s < t_steps
                has2 = s >= 1
                if j == 0:
                    seq_prev_tile, seq_cur = seq_cur, seqp.tile(
                        [32, CH * 2 * BS], f32)
                    if has1:
                        cl = min(CH, t_steps - chunk * CH) * BS
                        tk = tkpool.tile([1, CH * BS], f32)
                        nc.sync.dma_start(
                            out=tk[:, 0:cl],
                            in_=tok[:, chunk * CH * BS:chunk * CH * BS + cl])
                        oh_cur = ohpool.tile([VOCAB, CH * BS], f32)
                        nb = cl // 512
                        for blk in range(nb):
                            tb = tbps.tile([VOCAB, 512], f32)
                            nc.tensor.matmul(
                                tb[:], ones_t[:, 0:VOCAB],
                                tk[:, blk * 512:(blk + 1) * 512],
                                start=True, stop=True)
                            nc.vector.tensor_scalar(
                                out=oh_cur[:, blk * 512:(blk + 1) * 512],
                                in0=tb[:], scalar1=iota_t[:, 0:1],
                                scalar2=None, op0=A.is_equal)

                # previous superstep's state APs
                if s >= 1:
                    pj = (s - 1) % CH
                    ptile = seq_cur if j != 0 else seq_prev_tile
                    h1_prev = ptile[0:HID, pj * 2 * BS:pj * 2 * BS + BS]
                    h2_prev = ptile[0:HID, pj * 2 * BS + BS:(pj + 1) * 2 * BS]

                g_ps = gps.tile([G4H, 2 * BS], f32)
                if has1:
                    nc.tensor.matmul(g_ps[:, 0:BS], b1_t[:], ones_t[:, 0:BS],
                                     start=True, stop=False)
                    nc.tensor.matmul(g_ps[:, 0:BS], mx_t[:],
                                     oh_cur[:, j * BS:(j + 1) * BS],
                                     start=False, stop=(s == 0))
                    if s >= 1:
                        nc.tensor.matmul(g_ps[:, 0:BS], w1h_t[:], h1_prev,
                                         start=False, stop=True)
                if has2:
                    nc.tensor.matmul(g_ps[:, BS:2 * BS], b2_t[:],
                                     ones_t[:, 0:BS], start=True, stop=False)
                    nc.tensor.matmul(g_ps[:, BS:2 * BS], w2ih_t[:], h1_prev,
                                     start=False, stop=(s == 1))
                    if s >= 2:
                        nc.tensor.matmul(g_ps[:, BS:2 * BS], w2hh_t[:],
                                         h2_prev, start=False, stop=True)

                lo = 0 if has1 else BS
                hi = 2 * BS if has2 else BS
                cs = slice(lo, hi)

                # sigmoid of i/f/o blocks in place in PSUM (keeps one PSUM
                # operand in each DVE op below — exempts the equal-base
                # rule); g block to SBUF (rows 96:126, base-aligned).
                st = spool.tile([G4H, 2 * BS], f32)
                s_ps = sps.tile([96, 2 * BS], f32)
                nc.scalar.activation(s_ps[0:96, cs], g_ps[0:96, cs],
                                     AF.Sigmoid)
                nc.scalar.activation(st[96:126, cs], g_ps[96:126, cs],
                                     AF.Sigmoid)

                c_new = cc.tile([32, 2 * BS], f32)
                if s == 0:
                    # c1[0] = s_i*(p-1/2); c2[-1] = 0
                    nc.vector.scalar_tensor_tensor(
                        out=c_new[0:HID, 0:BS], in0=st[96:126, 0:BS],
                        scalar=-0.5, in1=s_ps[0:HID, 0:BS],
                        op0=A.add, op1=A.mult)
                    nc.vector.memset(c_new[0:HID, BS:2 * BS], 0.0)
                else:
                    r2 = r2p.tile([32, 2 * BS], f32)
                    nc.vector.scalar_tensor_tensor(
                        out=r2[0:HID, cs], in0=st[96:126, cs], scalar=-0.5,
                        in1=s_ps[0:HID, cs], op0=A.add, op1=A.mult)
                    fc = fcp.tile([32, 2 * BS], f32)
                    nc.vector.tensor_tensor(
                        out=fc[0:HID, cs], in0=s_ps[32:62, cs],
                        in1=c_prev[0:HID, cs], op=A.mult)
                    nc.vector.tensor_tensor(
                        out=c_new[0:HID, cs], in0=fc[0:HID, cs],
                        in1=r2[0:HID, cs], op=A.add)
                c_prev = c_new

                u = up.tile([32, 2 * BS], f32)
                nc.scalar.activation(u[0:HID, cs], c_new[0:HID, cs],
                                     AF.Sigmoid, scale=4.0)
                # h_hat -> current seq block j
                nc.vector.scalar_tensor_tensor(
                    out=seq_cur[0:HID, j * 2 * BS + lo:j * 2 * BS + hi],
                    in0=u[0:HID, cs], scalar=-0.5, in1=s_ps[64:94, cs],
                    op0=A.add, op1=A.mult)

                # head: after block 8g+7 of each chunk is written
                if has2 and (j % 8 == 7):
                    grp = j // 8
                    blk0, nblk = grp * 8, 8
                    t0 = chunk * CH + blk0 - 1
                    if chunk == 0 and grp == 0:
                        blk0, nblk, t0 = 1, 7, 0
                    head_group(seq_cur, blk0, nblk, t0)
                if s == t_steps:
                    # flush: ĥ2[t-1] sits in block j of current chunk
                    head_group(seq_cur, j, 1, s - 1)

    nc.compile()
    return nc


def _get_program():
    global _PROGRAM
    if _PROGRAM is None:
        _PROGRAM = build_program(T)
    return _PROGRAM


def _prep_weights(inp):
    f = lambda k: np.asarray(inp[k], np.float32)
    emb = f("embedding")
    mx = _expand_gate_cols(emb @ f("w_ih1").T)                  # [100,128]
    b1 = _expand_gate_cols((f("b_ih1") + f("b_hh1")))[None, :]  # [1,128]
    w1h = _expand_gate_cols(2.0 * f("w_hh1").T)                 # [30,128]
    w2ih = _expand_gate_cols(2.0 * f("w_ih2").T)
    w2hh = _expand_gate_cols(2.0 * f("w_hh2").T)
    b2 = _expand_gate_cols((f("b_ih2") + f("b_hh2")))[None, :]
    hw1 = np.ascontiguousarray((2.0 * f("lw1")).T)              # [30,30]
    hw2 = np.ascontiguousarray(f("lw2").T)                      # [30,100]
    lb1 = f("lb1")[:, None]
    lb2 = f("lb2")[:, None]
    iota = np.arange(VOCAB, dtype=np.float32)[:, None]
    ones = np.ones((1, 512), np.float32)
    return dict(mx=mx, b1=b1, w1h=w1h, w2ih=w2ih, w2hh=w2hh, b2=b2,
                hw1=hw1, hw2=hw2, lb1=lb1, lb2=lb2, iota=iota, ones=ones)


def kernel(**inputs):
    from concourse.bass_utils import run_bass_kernel_spmd

    nc = _get_program()
    batch_x = np.asarray(inputs["batch_x"])
    lens = np.asarray(inputs["batch_x_lens"]).astype(np.int64)
    consts = _prep_weights(inputs)

    in_maps = []
    for m in range(NCORES):
        toks = batch_x[m * BS:(m + 1) * BS].astype(np.float32)  # [BS, T]
        tok_flat = np.ascontiguousarray(toks.T).reshape(1, T * BS)
        in_maps.append({"tok": tok_flat, **consts})

    res = run_bass_kernel_spmd(nc, in_maps, list(range(NCORES)))
    outs = []
    for m in range(NCORES):
        o = res.results[m]["out"]                 # [100, T*BS]
        o = o.reshape(VOCAB, T, BS).transpose(2, 1, 0)  # [BS, T, 100]
        outs.append(o)
    full = np.concatenate(outs, axis=0)           # [B, T, 100]
    # reference zeroes h2 BEFORE the head, so masked positions equal the
    # constant head(0) = relu(lb1) @ lw2.T + lb2
    lb1 = np.asarray(inputs["lb1"], np.float32)
    lw2 = np.asarray(inputs["lw2"], np.float32)
    lb2 = np.asarray(inputs["lb2"], np.float32)
    head0 = np.maximum(lb1, 0.0) @ lw2.T + lb2    # [100]
    mask = np.arange(T)[None, :] < lens[:, None]
    full = np.where(mask[:, :, None], full, head0[None, None, :])
    return np.ascontiguousarray(full.astype(np.float32))


if __name__ == "__main__":
    ref = {}
    exec(open(os.path.join(os.path.dirname(__file__), "reference.py")).read(),
         ref)
    inputs = {k: np.asarray(v) for k, v in ref["setup_inputs"]().items()}
    got = kernel(**inputs)
    want = np.asarray(ref["reference"](**inputs))
    err = np.abs(got - want).max() / max(np.abs(want).max(), 1e-9)
    print("rel err:", err)


# revision 11
# speedup vs baseline: 1.4883x; 1.4883x over previous
"""CharRNN (2-layer masked LSTM + MLP head) Trainium2 Bass kernel.

Strategy: pure data parallel over batch (512 seqs -> 8 cores x 64).
Per core, a staggered 2-layer LSTM scan: superstep s computes layer-1
step s and layer-2 step s-1 side by side in 128-wide tiles
(64 batch cols x 2 layers).  Key transformations:

- Masked-LSTM semantics: because the mask (t < len) is monotone and the
  final output is zeroed where mask==0, running the scan UNMASKED gives
  bit-identical results in the valid region; masking reduces to zeroing
  the final logits (done host-side after gather).
- Embedding lookup + input projection fused: one matmul against a
  per-chunk one-hot matrix, with lhsT = emb @ w_ih1.T precomputed on
  host (tiny).  One-hot built on device: K=1 matmul broadcasts the
  token row across 100 partitions, then tensor_scalar(is_equal) against
  an iota column.
- Single sigmoid for all 4 gates: tanh(x) = 2*sigmoid(2x) - 1, with the
  x2 folded into the g-gate rows of all weight matrices.  State is kept
  scaled: h_hat = h/2, c_hat = c/2, with the compensating x2 folded into
  every matmul that consumes h (w_hh1, w_ih2, w_hh2, lw1).
- Cell update in 3 DVE ops (scalar_tensor_tensor fusions):
    r2 = (p - 0.5) * s_i          (= s_i*tanh(g)/2)
    fc = s_f * c_hat_prev
    c_hat = fc + r2
    u  = sigmoid(4*c_hat)          (ACT, scale=4 is free)
    h_hat = (u - 0.5) * s_o        (= h/2)
- Biases enter via K=1 rank-1 matmuls (bias_row (x) ones_row).
- Gate rows live in 32-aligned blocks (i:0-29, f:32-61, o:64-93,
  g:96-125) so every engine slice has a legal start partition.
- MLP head + output DMA pipelined per 8 supersteps, overlapped with the
  scan.  Device emits logits as [vocab, T*64]; host transposes/masks.
"""

import os
import sys

import numpy as np

sys.path.insert(0, "/opt/trn_rl_repo")

VOCAB, EMBED, HID, B, T = 100, 50, 30, 512, 512
NCORES = 8
BS = B // NCORES            # 64 sequences per core
CH = 64                     # supersteps per chunk
G4H = 128                   # padded gate dim (4 blocks of 32)

_PROGRAM = None             # (nc, input_names) cache


def _gate_perm():
    """Map torch gate-row order [i,f,g,o] (each 30) to 32-aligned blocks
    i->0:30, f->32:62, o->64:94, g->96:126 inside a 128-row layout."""
    perm = np.zeros(4 * HID, dtype=np.int64)
    perm[0:30] = np.arange(30)            # i
    perm[30:60] = 32 + np.arange(30)      # f
    perm[60:90] = 96 + np.arange(30)      # g
    perm[90:120] = 64 + np.arange(30)     # o
    return perm


def _expand_gate_cols(w):
    """w: [..., 120] gate-last -> [..., 128] permuted, g-cols doubled."""
    perm = _gate_perm()
    out = np.zeros(w.shape[:-1] + (G4H,), np.float32)
    out[..., perm] = w
    out[..., 96:126] *= 2.0
    return out


def build_program(t_steps=T):
    import concourse.bass as bass
    import concourse.bacc as bacc
    import concourse.mybir as mybir
    import concourse.tile as tile

    f32 = mybir.dt.float32
    f16 = mybir.dt.float16
    A = mybir.AluOpType
    AF = mybir.ActivationFunctionType

    n_chunks = (t_steps + CH - 1) // CH
    ncols = t_steps * BS

    nc = bacc.Bacc()

    tok = nc.declare_dram_parameter("tok", [1, ncols], f32, isOutput=False)
    mx = nc.declare_dram_parameter("mx", [VOCAB, G4H], f16, isOutput=False)
    w1h = nc.declare_dram_parameter("w1h", [HID, G4H], f16, isOutput=False)
    w2ih = nc.declare_dram_parameter("w2ih", [HID, G4H], f16, isOutput=False)
    w2hh = nc.declare_dram_parameter("w2hh", [HID, G4H], f16, isOutput=False)
    b1 = nc.declare_dram_parameter("b1", [1, G4H], f16, isOutput=False)
    b2 = nc.declare_dram_parameter("b2", [1, G4H], f16, isOutput=False)
    hw1 = nc.declare_dram_parameter("hw1", [HID, HID], f16, isOutput=False)
    hw2 = nc.declare_dram_parameter("hw2", [HID, VOCAB], f32, isOutput=False)
    lb1 = nc.declare_dram_parameter("lb1", [HID, 1], f32, isOutput=False)
    lb2 = nc.declare_dram_parameter("lb2", [VOCAB, 1], f32, isOutput=False)
    iota = nc.declare_dram_parameter("iota", [VOCAB, 1], f32, isOutput=False)
    ones = nc.declare_dram_parameter("ones", [1, 512], f32, isOutput=False)
    ones16 = nc.declare_dram_parameter("ones16", [1, 512], f16, isOutput=False)
    out = nc.declare_dram_parameter("out", [VOCAB, ncols], f32, isOutput=True)

    with tile.TileContext(nc) as tc:
        from contextlib import ExitStack
        with ExitStack() as ctx:
            cpool = ctx.enter_context(tc.tile_pool(name="consts", bufs=1))
            ohpool = ctx.enter_context(tc.tile_pool(name="oh", bufs=2))
            tkpool = ctx.enter_context(tc.tile_pool(name="tk", bufs=2))
            tbps = ctx.enter_context(
                tc.tile_pool(name="tbps", bufs=1, space="PSUM"))
            gps = ctx.enter_context(
                tc.tile_pool(name="gps", bufs=3, space="PSUM"))
            sps = ctx.enter_context(
                tc.tile_pool(name="sps", bufs=2, space="PSUM"))
            spool = ctx.enter_context(tc.tile_pool(name="sg", bufs=3))
            cc = ctx.enter_context(tc.tile_pool(name="cc", bufs=3))
            r2p = ctx.enter_context(tc.tile_pool(name="r2", bufs=3))
            fcp = ctx.enter_context(tc.tile_pool(name="fcb", bufs=3))
            up = ctx.enter_context(tc.tile_pool(name="ub", bufs=3))
            seqp = ctx.enter_context(tc.tile_pool(name="seq", bufs=2))
            hmps = ctx.enter_context(
                tc.tile_pool(name="hmps", bufs=1, space="PSUM"))
            hmp = ctx.enter_context(tc.tile_pool(name="hm", bufs=2))
            lgps = ctx.enter_context(
                tc.tile_pool(name="lgps", bufs=1, space="PSUM"))
            lgp = ctx.enter_context(tc.tile_pool(name="lg", bufs=3))

            def load_const(ap, shape, tag, dt=f32):
                t = cpool.tile(list(shape), dt, tag=tag)
                nc.sync.dma_start(out=t[:], in_=ap[:])
                return t

            mx_t = load_const(mx, (VOCAB, G4H), "mx", f16)
            w1h_t = load_const(w1h, (HID, G4H), "w1h", f16)
            w2ih_t = load_const(w2ih, (HID, G4H), "w2ih", f16)
            w2hh_t = load_const(w2hh, (HID, G4H), "w2hh", f16)
            b1_t = load_const(b1, (1, G4H), "b1", f16)
            b2_t = load_const(b2, (1, G4H), "b2", f16)
            hw1_t = load_const(hw1, (HID, HID), "hw1", f16)
            hw2_t = load_const(hw2, (HID, VOCAB), "hw2")
            lb1_t = load_const(lb1, (HID, 1), "lb1")
            lb2_t = load_const(lb2, (VOCAB, 1), "lb2")
            iota_t = load_const(iota, (VOCAB, 1), "iota")
            ones_t = load_const(ones, (1, 512), "ones")
            ones16_t = load_const(ones16, (1, 512), "ones16", f16)

            c_prev = None           # AP of previous c_hat tile
            seq_cur = None          # current chunk h-seq tile
            seq_prev_tile = None    # previous chunk h-seq tile
            oh_cur = None

            def head_group(seq_tile, blk0, nblk, t0):
                """MLP head over ĥ2 blocks [blk0, blk0+nblk) of seq_tile;
                t0 = timestep of block blk0; writes out cols t0*BS.."""
                n = nblk * BS
                r = seq_tile[0:HID, :].rearrange(
                    "p (j c) -> p j c", c=2 * BS)
                rhs = r[:, blk0:blk0 + nblk, BS:2 * BS]
                hm_ps = hmps.tile([HID, 512], f32)
                nc.tensor.matmul(hm_ps[:, 0:n], hw1_t[:], rhs,
                                 start=True, stop=True)
                hm = hmp.tile([32, 512], f32)
                nc.scalar.activation(hm[0:HID, 0:n], hm_ps[0:HID, 0:n],
                                     AF.Relu, bias=lb1_t[:, 0:1])
                lg_ps = lgps.tile([VOCAB, 512], f32)
                nc.tensor.matmul(lg_ps[:, 0:n], hw2_t[:], hm[0:HID, 0:n],
                                 start=True, stop=True)
                lg = lgp.tile([VOCAB, 512], f32)
                nc.scalar.activation(lg[:, 0:n], lg_ps[:, 0:n],
                                     AF.Identity, bias=lb2_t[:, 0:1])
                nc.sync.dma_start(out=out[:, t0 * BS:t0 * BS + n],
                                  in_=lg[:, 0:n])

            for s in range(t_steps + 1):
                chunk, j = s // CH, s % CH
                has1 = s < t_steps
                has2 = s >= 1
                if j == 0:
                    seq_prev_tile, seq_cur = seq_cur, seqp.tile(
                        [32, CH * 2 * BS], f16)
                    if has1:
                        cl = min(CH, t_steps - chunk * CH) * BS
                        tk = tkpool.tile([1, CH * BS], f32)
                        nc.sync.dma_start(
                            out=tk[:, 0:cl],
                            in_=tok[:, chunk * CH * BS:chunk * CH * BS + cl])
                        oh_cur = ohpool.tile([VOCAB, CH * BS], f16)
                        nb = cl // 512
                        for blk in range(nb):
                            tb = tbps.tile([VOCAB, 512], f32)
                            nc.tensor.matmul(
                                tb[:], ones_t[:, 0:VOCAB],
                                tk[:, blk * 512:(blk + 1) * 512],
                                start=True, stop=True)
                            nc.vector.tensor_scalar(
                                out=oh_cur[:, blk * 512:(blk + 1) * 512],
                                in0=tb[:], scalar1=iota_t[:, 0:1],
                                scalar2=None, op0=A.is_equal)

                # previous superstep's state APs
                if s >= 1:
                    pj = (s - 1) % CH
                    ptile = seq_cur if j != 0 else seq_prev_tile
                    h1_prev = ptile[0:HID, pj * 2 * BS:pj * 2 * BS + BS]
                    h2_prev = ptile[0:HID, pj * 2 * BS + BS:(pj + 1) * 2 * BS]

                g_ps = gps.tile([G4H, 2 * BS], f32)
                if has1:
                    nc.tensor.matmul(g_ps[:, 0:BS], b1_t[:], ones16_t[:, 0:BS],
                                     start=True, stop=False)
                    nc.tensor.matmul(g_ps[:, 0:BS], mx_t[:],
                                     oh_cur[:, j * BS:(j + 1) * BS],
                                     start=False, stop=(s == 0))
                    if s >= 1:
                        nc.tensor.matmul(g_ps[:, 0:BS], w1h_t[:], h1_prev,
                                         start=False, stop=True)
                if has2:
                    nc.tensor.matmul(g_ps[:, BS:2 * BS], b2_t[:],
                                     ones16_t[:, 0:BS], start=True, stop=False)
                    nc.tensor.matmul(g_ps[:, BS:2 * BS], w2ih_t[:], h1_prev,
                                     start=False, stop=(s == 1))
                    if s >= 2:
                        nc.tensor.matmul(g_ps[:, BS:2 * BS], w2hh_t[:],
                                         h2_prev, start=False, stop=True)

                lo = 0 if has1 else BS
                hi = 2 * BS if has2 else BS
                cs = slice(lo, hi)

                # sigmoid of i/f/o blocks in place in PSUM (keeps one PSUM
                # operand in each DVE op below — exempts the equal-base
                # rule); g block to SBUF (rows 96:126, base-aligned).
                st = spool.tile([G4H, 2 * BS], f32)
                s_ps = sps.tile([96, 2 * BS], f32)
                nc.scalar.activation(s_ps[0:96, cs], g_ps[0:96, cs],
                                     AF.Sigmoid)
                nc.scalar.activation(st[96:126, cs], g_ps[96:126, cs],
                                     AF.Sigmoid)

                c_new = cc.tile([32, 2 * BS], f32)
                if s == 0:
                    # c1[0] = s_i*(p-1/2); c2[-1] = 0
                    nc.vector.scalar_tensor_tensor(
                        out=c_new[0:HID, 0:BS], in0=st[96:126, 0:BS],
                        scalar=-0.5, in1=s_ps[0:HID, 0:BS],
                        op0=A.add, op1=A.mult)
                    nc.vector.memset(c_new[0:HID, BS:2 * BS], 0.0)
                else:
                    r2 = r2p.tile([32, 2 * BS], f32)
                    nc.vector.scalar_tensor_tensor(
                        out=r2[0:HID, cs], in0=st[96:126, cs], scalar=-0.5,
                        in1=s_ps[0:HID, cs], op0=A.add, op1=A.mult)
                    fc = fcp.tile([32, 2 * BS], f32)
                    nc.vector.tensor_tensor(
                        out=fc[0:HID, cs], in0=s_ps[32:62, cs],
                        in1=c_prev[0:HID, cs], op=A.mult)
                    nc.vector.tensor_tensor(
                        out=c_new[0:HID, cs], in0=fc[0:HID, cs],
                        in1=r2[0:HID, cs], op=A.add)
                c_prev = c_new

                u = up.tile([32, 2 * BS], f32)
                nc.scalar.activation(u[0:HID, cs], c_new[0:HID, cs],
                                     AF.Sigmoid, scale=4.0)
                # h_hat -> current seq block j
                nc.vector.scalar_tensor_tensor(
                    out=seq_cur[0:HID, j * 2 * BS + lo:j * 2 * BS + hi],
                    in0=u[0:HID, cs], scalar=-0.5, in1=s_ps[64:94, cs],
                    op0=A.add, op1=A.mult)

                # head: after block 8g+7 of each chunk is written
                if has2 and (j % 8 == 7):
                    grp = j // 8
                    blk0, nblk = grp * 8, 8
                    t0 = chunk * CH + blk0 - 1
                    if chunk == 0 and grp == 0:
                        blk0, nblk, t0 = 1, 7, 0
                    head_group(seq_cur, blk0, nblk, t0)
                if s == t_steps:
                    # flush: ĥ2[t-1] sits in block j of current chunk
                    head_group(seq_cur, j, 1, s - 1)

    nc.compile()
    return nc


def _get_program():
    global _PROGRAM
    if _PROGRAM is None:
        _PROGRAM = build_program(T)
    return _PROGRAM


def _prep_weights(inp):
    f = lambda k: np.asarray(inp[k], np.float32)
    emb = f("embedding")
    mx = _expand_gate_cols(emb @ f("w_ih1").T)                  # [100,128]
    b1 = _expand_gate_cols((f("b_ih1") + f("b_hh1")))[None, :]  # [1,128]
    w1h = _expand_gate_cols(2.0 * f("w_hh1").T)                 # [30,128]
    w2ih = _expand_gate_cols(2.0 * f("w_ih2").T)
    w2hh = _expand_gate_cols(2.0 * f("w_hh2").T)
    b2 = _expand_gate_cols((f("b_ih2") + f("b_hh2")))[None, :]
    hw1 = np.ascontiguousarray((2.0 * f("lw1")).T)              # [30,30]
    hw2 = np.ascontiguousarray(f("lw2").T)                      # [30,100]
    lb1 = f("lb1")[:, None]
    lb2 = f("lb2")[:, None]
    iota = np.arange(VOCAB, dtype=np.float32)[:, None]
    ones = np.ones((1, 512), np.float32)
    h = np.float16
    return dict(mx=mx.astype(h), b1=b1.astype(h), w1h=w1h.astype(h),
                w2ih=w2ih.astype(h), w2hh=w2hh.astype(h), b2=b2.astype(h),
                hw1=hw1.astype(h), hw2=hw2, lb1=lb1, lb2=lb2, iota=iota,
                ones=ones, ones16=np.ones((1, 512), h))


def kernel(**inputs):
    from concourse.bass_utils import run_bass_kernel_spmd

    nc = _get_program()
    batch_x = np.asarray(inputs["batch_x"])
    lens = np.asarray(inputs["batch_x_lens"]).astype(np.int64)
    consts = _prep_weights(inputs)

    in_maps = []
    for m in range(NCORES):
        toks = batch_x[m * BS:(m + 1) * BS].astype(np.float32)  # [BS, T]
        tok_flat = np.ascontiguousarray(toks.T).reshape(1, T * BS)
        in_maps.append({"tok": tok_flat, **consts})

    res = run_bass_kernel_spmd(nc, in_maps, list(range(NCORES)))
    outs = []
    for m in range(NCORES):
        o = res.results[m]["out"]                 # [100, T*BS]
        o = o.reshape(VOCAB, T, BS).transpose(2, 1, 0)  # [BS, T, 100]
        outs.append(o)
    full = np.concatenate(outs, axis=0)           # [B, T, 100]
    # reference zeroes h2 BEFORE the head, so masked positions equal the
    # constant head(0) = relu(lb1) @ lw2.T + lb2
    lb1 = np.asarray(inputs["lb1"], np.float32)
    lw2 = np.asarray(inputs["lw2"], np.float32)
    lb2 = np.asarray(inputs["lb2"], np.float32)
    head0 = np.maximum(lb1, 0.0) @ lw2.T + lb2    # [100]
    mask = np.arange(T)[None, :] < lens[:, None]
    full = np.where(mask[:, :, None], full, head0[None, None, :])
    return np.ascontiguousarray(full.astype(np.float32))


if __name__ == "__main__":
    ref = {}
    exec(open(os.path.join(os.path.dirname(__file__), "reference.py")).read(),
         ref)
    inputs = {k: np.asarray(v) for k, v in ref["setup_inputs"]().items()}
    got = kernel(**inputs)
    want = np.asarray(ref["reference"](**inputs))
    err = np.abs(got - want).max() / max(np.abs(want).max(), 1e-9)
    print("rel err:", err)


# revision 14
# speedup vs baseline: 1.5599x; 1.0481x over previous
"""CharRNN (2-layer masked LSTM + MLP head) Trainium2 Bass kernel.

Strategy: pure data parallel over batch (512 seqs -> 8 cores x 64).
Per core, a staggered 2-layer LSTM scan: superstep s computes layer-1
step s and layer-2 step s-1 side by side in 128-wide tiles
(64 batch cols x 2 layers).  Key transformations:

- Masked-LSTM semantics: because the mask (t < len) is monotone and the
  final output is zeroed where mask==0, running the scan UNMASKED gives
  bit-identical results in the valid region; masking reduces to zeroing
  the final logits (done host-side after gather).
- Embedding lookup + input projection fused: one matmul against a
  per-chunk one-hot matrix, with lhsT = emb @ w_ih1.T precomputed on
  host (tiny).  One-hot built on device: K=1 matmul broadcasts the
  token row across 100 partitions, then tensor_scalar(is_equal) against
  an iota column.
- Single sigmoid for all 4 gates: tanh(x) = 2*sigmoid(2x) - 1, with the
  x2 folded into the g-gate rows of all weight matrices.  State is kept
  scaled: h_hat = h/2, c_hat = c/2, with the compensating x2 folded into
  every matmul that consumes h (w_hh1, w_ih2, w_hh2, lw1).
- Cell update in 3 DVE ops (scalar_tensor_tensor fusions):
    r2 = (p - 0.5) * s_i          (= s_i*tanh(g)/2)
    fc = s_f * c_hat_prev
    c_hat = fc + r2
    u  = sigmoid(4*c_hat)          (ACT, scale=4 is free)
    h_hat = (u - 0.5) * s_o        (= h/2)
- Biases enter via K=1 rank-1 matmuls (bias_row (x) ones_row).
- Gate rows live in 32-aligned blocks (i:0-29, f:32-61, o:64-93,
  g:96-125) so every engine slice has a legal start partition.
- MLP head + output DMA pipelined per 8 supersteps, overlapped with the
  scan.  Device emits logits as [vocab, T*64]; host transposes/masks.
"""

import os
import sys

import numpy as np

sys.path.insert(0, "/opt/trn_rl_repo")

VOCAB, EMBED, HID, B, T = 100, 50, 30, 512, 512
NCORES = 8
BS = B // NCORES            # 64 sequences per core
CH = 64                     # supersteps per chunk
G4H = 128                   # padded gate dim (4 blocks of 32)

_PROGRAM = None             # (nc, input_names) cache


def _gate_perm():
    """Map torch gate-row order [i,f,g,o] (each 30) to 32-aligned blocks
    i->0:30, o->32:62, f->64:94, g->96:126 inside a 128-row layout."""
    perm = np.zeros(4 * HID, dtype=np.int64)
    perm[0:30] = np.arange(30)            # i
    perm[30:60] = 64 + np.arange(30)      # f
    perm[60:90] = 96 + np.arange(30)      # g
    perm[90:120] = 32 + np.arange(30)     # o
    return perm


def _expand_gate_cols(w):
    """w: [..., 120] gate-last -> [..., 128] permuted, g-cols doubled."""
    perm = _gate_perm()
    out = np.zeros(w.shape[:-1] + (G4H,), np.float32)
    out[..., perm] = w
    out[..., 96:126] *= 2.0
    return out


def build_program(t_steps=T):
    import concourse.bass as bass
    import concourse.bacc as bacc
    import concourse.mybir as mybir
    import concourse.tile as tile

    f32 = mybir.dt.float32
    f16 = mybir.dt.float16
    f32r = mybir.dt.float32r
    A = mybir.AluOpType
    AF = mybir.ActivationFunctionType

    n_chunks = (t_steps + CH - 1) // CH
    ncols = t_steps * BS

    nc = bacc.Bacc()

    tok = nc.declare_dram_parameter("tok", [1, ncols], f16, isOutput=False)
    mx = nc.declare_dram_parameter("mx", [VOCAB, G4H], f16, isOutput=False)
    w1h = nc.declare_dram_parameter("w1h", [HID, G4H], f16, isOutput=False)
    w2ih = nc.declare_dram_parameter("w2ih", [HID, G4H], f16, isOutput=False)
    w2hh = nc.declare_dram_parameter("w2hh", [HID, G4H], f16, isOutput=False)
    b1 = nc.declare_dram_parameter("b1", [1, G4H], f16, isOutput=False)
    b2 = nc.declare_dram_parameter("b2", [1, G4H], f16, isOutput=False)
    hw1 = nc.declare_dram_parameter("hw1", [HID, HID], f16, isOutput=False)
    hw2 = nc.declare_dram_parameter("hw2", [HID, VOCAB], f32, isOutput=False)
    lb1 = nc.declare_dram_parameter("lb1", [HID, 1], f32, isOutput=False)
    lb2 = nc.declare_dram_parameter("lb2", [VOCAB, 1], f32, isOutput=False)
    iota = nc.declare_dram_parameter("iota", [VOCAB, 1], f32, isOutput=False)
    ones = nc.declare_dram_parameter("ones", [1, 512], f32, isOutput=False)
    ones16 = nc.declare_dram_parameter("ones16", [1, 512], f16, isOutput=False)
    out = nc.declare_dram_parameter("out", [VOCAB, ncols], f32, isOutput=True)

    with tile.TileContext(nc) as tc:
        from contextlib import ExitStack
        with ExitStack() as ctx:
            cpool = ctx.enter_context(tc.tile_pool(name="consts", bufs=1))
            ohpool = ctx.enter_context(tc.tile_pool(name="oh", bufs=2))
            tkpool = ctx.enter_context(tc.tile_pool(name="tk", bufs=2))
            tbps = ctx.enter_context(
                tc.tile_pool(name="tbps", bufs=1, space="PSUM"))
            gps = ctx.enter_context(
                tc.tile_pool(name="gps", bufs=2, space="PSUM"))
            sps = ctx.enter_context(
                tc.tile_pool(name="sps", bufs=1, space="PSUM"))
            spool = ctx.enter_context(tc.tile_pool(name="sg", bufs=3))
            cc = ctx.enter_context(
                tc.tile_pool(name="cc", bufs=2, space="PSUM"))
            r2p = ctx.enter_context(tc.tile_pool(name="r2", bufs=3))
            fcp = ctx.enter_context(tc.tile_pool(name="fcb", bufs=3))
            up = ctx.enter_context(tc.tile_pool(name="ub", bufs=3))
            seqp = ctx.enter_context(tc.tile_pool(name="seq", bufs=2))
            hmps = ctx.enter_context(
                tc.tile_pool(name="hmps", bufs=1, space="PSUM"))
            hmp = ctx.enter_context(tc.tile_pool(name="hm", bufs=2))
            lgps = ctx.enter_context(
                tc.tile_pool(name="lgps", bufs=1, space="PSUM"))
            lgp = ctx.enter_context(tc.tile_pool(name="lg", bufs=3))

            def load_const(ap, shape, tag, dt=f32):
                t = cpool.tile(list(shape), dt, tag=tag)
                nc.sync.dma_start(out=t[:], in_=ap[:])
                return t

            mx_t = load_const(mx, (VOCAB, G4H), "mx", f16)
            w1h_t = load_const(w1h, (HID, G4H), "w1h", f16)
            w2ih_t = load_const(w2ih, (HID, G4H), "w2ih", f16)
            w2hh_t = load_const(w2hh, (HID, G4H), "w2hh", f16)
            b1_t = load_const(b1, (1, G4H), "b1", f16)
            b2_t = load_const(b2, (1, G4H), "b2", f16)
            hw1_t = load_const(hw1, (HID, HID), "hw1", f16)
            hw2_t = load_const(hw2, (HID, VOCAB), "hw2")
            lb1_t = load_const(lb1, (HID, 1), "lb1")
            lb2_t = load_const(lb2, (VOCAB, 1), "lb2")
            iota_t = load_const(iota, (VOCAB, 1), "iota")
            ones_t = load_const(ones, (1, 512), "ones")
            ones16_t = load_const(ones16, (1, 512), "ones16", f16)

            c_prev = None           # AP of previous c_hat tile
            seq_cur = None          # current chunk h-seq tile
            seq_prev_tile = None    # previous chunk h-seq tile
            oh_cur = None

            def head_group(seq_tile, blk0, nblk, t0):
                """MLP head over ĥ2 blocks [blk0, blk0+nblk) of seq_tile;
                t0 = timestep of block blk0; writes out cols t0*BS.."""
                n = nblk * BS
                r = seq_tile[0:HID, :].rearrange(
                    "p (j c) -> p j c", c=2 * BS)
                rhs = r[:, blk0:blk0 + nblk, BS:2 * BS]
                hm_ps = hmps.tile([HID, 512], f32)
                nc.tensor.matmul(hm_ps[:, 0:n], hw1_t[:], rhs,
                                 start=True, stop=True)
                hm = hmp.tile([32, 512], f32)
                nc.scalar.activation(hm[0:HID, 0:n], hm_ps[0:HID, 0:n],
                                     AF.Relu, bias=lb1_t[:, 0:1])
                lg_ps = lgps.tile([VOCAB, 512], f32)
                nc.tensor.matmul(lg_ps[:, 0:n], hw2_t[:], hm[0:HID, 0:n],
                                 start=True, stop=True)
                lg = lgp.tile([VOCAB, 512], f32)
                nc.scalar.activation(lg[:, 0:n], lg_ps[:, 0:n],
                                     AF.Identity, bias=lb2_t[:, 0:1])
                nc.sync.dma_start(out=out[:, t0 * BS:t0 * BS + n],
                                  in_=lg[:, 0:n])

            for s in range(t_steps + 1):
                chunk, j = s // CH, s % CH
                has1 = s < t_steps
                has2 = s >= 1
                if j == 0:
                    seq_prev_tile, seq_cur = seq_cur, seqp.tile(
                        [32, CH * 2 * BS], f16)
                    if has1:
                        cl = min(CH, t_steps - chunk * CH) * BS
                        tk = tkpool.tile([1, CH * BS], f16)
                        nc.sync.dma_start(
                            out=tk[:, 0:cl],
                            in_=tok[:, chunk * CH * BS:chunk * CH * BS + cl])
                        oh_cur = ohpool.tile([VOCAB, CH * BS], f16)
                        nb = cl // 512
                        for blk in range(nb):
                            tb = tbps.tile([VOCAB, 512], f32)
                            nc.tensor.matmul(
                                tb[:], ones16_t[:, 0:VOCAB],
                                tk[:, blk * 512:(blk + 1) * 512],
                                start=True, stop=True)
                            nc.vector.tensor_scalar(
                                out=oh_cur[:, blk * 512:(blk + 1) * 512],
                                in0=tb[:], scalar1=iota_t[:, 0:1],
                                scalar2=None, op0=A.is_equal)

                # previous superstep's state APs
                if s >= 1:
                    pj = (s - 1) % CH
                    ptile = seq_cur if j != 0 else seq_prev_tile
                    h1_prev = ptile[0:HID, pj * 2 * BS:pj * 2 * BS + BS]
                    h2_prev = ptile[0:HID, pj * 2 * BS + BS:(pj + 1) * 2 * BS]

                g_ps = gps.tile([G4H, 2 * BS], f32)
                if has1:
                    # b1 is folded into mx (one-hot columns sum to 1)
                    nc.tensor.matmul(g_ps[:, 0:BS], mx_t[:],
                                     oh_cur[:, j * BS:(j + 1) * BS],
                                     start=True, stop=(s == 0))
                    if s >= 1:
                        nc.tensor.matmul(g_ps[:, 0:BS], w1h_t[:], h1_prev,
                                         start=False, stop=True)
                if has2:
                    nc.tensor.matmul(g_ps[:, BS:2 * BS], b2_t[:],
                                     ones16_t[:, 0:BS], start=True, stop=False)
                    nc.tensor.matmul(g_ps[:, BS:2 * BS], w2ih_t[:], h1_prev,
                                     start=False, stop=(s == 1))
                    if s >= 2:
                        nc.tensor.matmul(g_ps[:, BS:2 * BS], w2hh_t[:],
                                         h2_prev, start=False, stop=True)

                lo = 0 if has1 else BS
                hi = 2 * BS if has2 else BS
                cs = slice(lo, hi)

                # gate blocks: i 0:30, o 32:62 -> PSUM s_ps; f 64:94,
                # g 96:126 -> SBUF st.  Every DVE op below mixes one PSUM
                # and one SBUF operand (exempts the equal-base rule).
                st = spool.tile([G4H, 2 * BS], f32)
                s_ps = sps.tile([64, 2 * BS], f32)
                nc.scalar.activation(s_ps[0:64, cs], g_ps[0:64, cs],
                                     AF.Sigmoid)
                nc.scalar.activation(st[64:126, cs], g_ps[64:126, cs],
                                     AF.Sigmoid)

                c_new = cc.tile([32, 2 * BS], f32)
                if s == 0:
                    # c1[0] = s_i*(p-1/2); c2[-1] = 0
                    nc.vector.scalar_tensor_tensor(
                        out=c_new[0:HID, 0:BS], in0=st[96:126, 0:BS],
                        scalar=-0.5, in1=s_ps[0:HID, 0:BS],
                        op0=A.add, op1=A.mult)
                    nc.vector.memset(c_new[0:HID, BS:2 * BS], 0.0)
                else:
                    r2 = r2p.tile([32, 2 * BS], f32)
                    nc.vector.scalar_tensor_tensor(
                        out=r2[0:HID, cs], in0=st[96:126, cs], scalar=-0.5,
                        in1=s_ps[0:HID, cs], op0=A.add, op1=A.mult)
                    fc = fcp.tile([32, 2 * BS], f32)
                    nc.vector.tensor_tensor(
                        out=fc[0:HID, cs], in0=st[64:94, cs],
                        in1=c_prev[0:HID, cs], op=A.mult)
                    nc.vector.tensor_tensor(
                        out=c_new[0:HID, cs], in0=fc[0:HID, cs],
                        in1=r2[0:HID, cs], op=A.add)
                c_prev = c_new

                u = up.tile([32, 2 * BS], f32)
                nc.scalar.activation(u[0:HID, cs], c_new[0:HID, cs],
                                     AF.Sigmoid, scale=4.0)
                # h_hat -> current seq block j
                nc.vector.scalar_tensor_tensor(
                    out=seq_cur[0:HID, j * 2 * BS + lo:j * 2 * BS + hi],
                    in0=u[0:HID, cs], scalar=-0.5, in1=s_ps[32:62, cs],
                    op0=A.add, op1=A.mult)

                # head: after block 8g+7 of each chunk is written
                if has2 and (j % 8 == 7):
                    grp = j // 8
                    blk0, nblk = grp * 8, 8
                    t0 = chunk * CH + blk0 - 1
                    if chunk == 0 and grp == 0:
                        blk0, nblk, t0 = 1, 7, 0
                    head_group(seq_cur, blk0, nblk, t0)
                if s == t_steps:
                    # flush: ĥ2[t-1] sits in block j of current chunk
                    head_group(seq_cur, j, 1, s - 1)

    nc.compile()
    return nc


def _get_program():
    global _PROGRAM
    if _PROGRAM is None:
        _PROGRAM = build_program(T)
    return _PROGRAM


def _prep_weights(inp):
    f = lambda k: np.asarray(inp[k], np.float32)
    emb = f("embedding")
    mx = _expand_gate_cols(emb @ f("w_ih1").T)                  # [100,128]
    b1 = _expand_gate_cols((f("b_ih1") + f("b_hh1")))[None, :]  # [1,128]
    mx = mx + b1   # one-hot columns sum to 1 -> adds b1 exactly
    w1h = _expand_gate_cols(2.0 * f("w_hh1").T)                 # [30,128]
    w2ih = _expand_gate_cols(2.0 * f("w_ih2").T)
    w2hh = _expand_gate_cols(2.0 * f("w_hh2").T)
    b2 = _expand_gate_cols((f("b_ih2") + f("b_hh2")))[None, :]
    hw1 = np.ascontiguousarray((2.0 * f("lw1")).T)              # [30,30]
    hw2 = np.ascontiguousarray(f("lw2").T)                      # [30,100]
    lb1 = f("lb1")[:, None]
    lb2 = f("lb2")[:, None]
    iota = np.arange(VOCAB, dtype=np.float32)[:, None]
    ones = np.ones((1, 512), np.float32)
    h = np.float16
    return dict(mx=mx.astype(h), b1=b1.astype(h), w1h=w1h.astype(h),
                w2ih=w2ih.astype(h), w2hh=w2hh.astype(h), b2=b2.astype(h),
                hw1=hw1.astype(h), hw2=hw2, lb1=lb1, lb2=lb2, iota=iota,
                ones=ones, ones16=np.ones((1, 512), h))


def kernel(**inputs):
    from concourse.bass_utils import run_bass_kernel_spmd

    nc = _get_program()
    batch_x = np.asarray(inputs["batch_x"])
    lens = np.asarray(inputs["batch_x_lens"]).astype(np.int64)
    consts = _prep_weights(inputs)

    in_maps = []
    for m in range(NCORES):
        toks = batch_x[m * BS:(m + 1) * BS].astype(np.float16)  # [BS, T]
        tok_flat = np.ascontiguousarray(toks.T).reshape(1, T * BS)
        in_maps.append({"tok": tok_flat, **consts})

    res = run_bass_kernel_spmd(nc, in_maps, list(range(NCORES)))
    outs = []
    for m in range(NCORES):
        o = res.results[m]["out"]                 # [100, T*BS]
        o = o.reshape(VOCAB, T, BS).transpose(2, 1, 0)  # [BS, T, 100]
        outs.append(o)
    full = np.concatenate(outs, axis=0)           # [B, T, 100]
    # reference zeroes h2 BEFORE the head, so masked positions equal the
    # constant head(0) = relu(lb1) @ lw2.T + lb2
    lb1 = np.asarray(inputs["lb1"], np.float32)
    lw2 = np.asarray(inputs["lw2"], np.float32)
    lb2 = np.asarray(inputs["lb2"], np.float32)
    head0 = np.maximum(lb1, 0.0) @ lw2.T + lb2    # [100]
    mask = np.arange(T)[None, :] < lens[:, None]
    full = np.where(mask[:, :, None], full, head0[None, None, :])
    return np.ascontiguousarray(full.astype(np.float32))


if __name__ == "__main__":
    ref = {}
    exec(open(os.path.join(os.path.dirname(__file__), "reference.py")).read(),
         ref)
    inputs = {k: np.asarray(v) for k, v in ref["setup_inputs"]().items()}
    got = kernel(**inputs)
    want = np.asarray(ref["reference"](**inputs))
    err = np.abs(got - want).max() / max(np.abs(want).max(), 1e-9)
    print("rel err:", err)


# revision 15
# speedup vs baseline: 1.6671x; 1.0687x over previous
"""CharRNN (2-layer masked LSTM + MLP head) Trainium2 Bass kernel.

Strategy: pure data parallel over batch (512 seqs -> 8 cores x 64).
Per core, a staggered 2-layer LSTM scan: superstep s computes layer-1
step s and layer-2 step s-1 side by side in 128-wide tiles
(64 batch cols x 2 layers).  Key transformations:

- Masked-LSTM semantics: because the mask (t < len) is monotone and the
  final output is zeroed where mask==0, running the scan UNMASKED gives
  bit-identical results in the valid region; masking reduces to zeroing
  the final logits (done host-side after gather).
- Embedding lookup + input projection fused: one matmul against a
  per-chunk one-hot matrix, with lhsT = emb @ w_ih1.T precomputed on
  host (tiny).  One-hot built on device: K=1 matmul broadcasts the
  token row across 100 partitions, then tensor_scalar(is_equal) against
  an iota column.
- Single sigmoid for all 4 gates: tanh(x) = 2*sigmoid(2x) - 1, with the
  x2 folded into the g-gate rows of all weight matrices.  State is kept
  scaled: h_hat = h/2, c_hat = c/2, with the compensating x2 folded into
  every matmul that consumes h (w_hh1, w_ih2, w_hh2, lw1).
- Cell update in 3 DVE ops (scalar_tensor_tensor fusions):
    r2 = (p - 0.5) * s_i          (= s_i*tanh(g)/2)
    fc = s_f * c_hat_prev
    c_hat = fc + r2
    u  = sigmoid(4*c_hat)          (ACT, scale=4 is free)
    h_hat = (u - 0.5) * s_o        (= h/2)
- Biases enter via K=1 rank-1 matmuls (bias_row (x) ones_row).
- Gate rows live in 32-aligned blocks (i:0-29, f:32-61, o:64-93,
  g:96-125) so every engine slice has a legal start partition.
- MLP head + output DMA pipelined per 8 supersteps, overlapped with the
  scan.  Device emits logits as [vocab, T*64]; host transposes/masks.
"""

import os
import sys

import numpy as np

sys.path.insert(0, "/opt/trn_rl_repo")

VOCAB, EMBED, HID, B, T = 100, 50, 30, 512, 512
NCORES = 8
BS = B // NCORES            # 64 sequences per core
CH = 64                     # supersteps per chunk
G4H = 128                   # padded gate dim (4 blocks of 32)

_PROGRAM = None             # (nc, input_names) cache


def _gate_perm():
    """Map torch gate-row order [i,f,g,o] (each 30) to 32-aligned blocks
    i->0:30, o->32:62, f->64:94, g->96:126 inside a 128-row layout."""
    perm = np.zeros(4 * HID, dtype=np.int64)
    perm[0:30] = np.arange(30)            # i
    perm[30:60] = 32 + np.arange(30)      # f
    perm[60:90] = 96 + np.arange(30)      # g
    perm[90:120] = 64 + np.arange(30)     # o
    return perm


def _expand_gate_cols(w):
    """w: [..., 120] gate-last -> [..., 128] permuted, g-cols doubled."""
    perm = _gate_perm()
    out = np.zeros(w.shape[:-1] + (G4H,), np.float32)
    out[..., perm] = w
    return out


def build_program(t_steps=T):
    import concourse.bass as bass
    import concourse.bacc as bacc
    import concourse.mybir as mybir
    import concourse.tile as tile

    f32 = mybir.dt.float32
    f16 = mybir.dt.float16
    f32r = mybir.dt.float32r
    A = mybir.AluOpType
    AF = mybir.ActivationFunctionType

    n_chunks = (t_steps + CH - 1) // CH
    ncols = t_steps * BS

    nc = bacc.Bacc()

    tok = nc.declare_dram_parameter("tok", [1, ncols], f16, isOutput=False)
    mx = nc.declare_dram_parameter("mx", [VOCAB, G4H], f16, isOutput=False)
    w1h = nc.declare_dram_parameter("w1h", [HID, G4H], f16, isOutput=False)
    w2ih = nc.declare_dram_parameter("w2ih", [HID, G4H], f16, isOutput=False)
    w2hh = nc.declare_dram_parameter("w2hh", [HID, G4H], f16, isOutput=False)
    b1 = nc.declare_dram_parameter("b1", [1, G4H], f16, isOutput=False)
    b2 = nc.declare_dram_parameter("b2", [1, G4H], f16, isOutput=False)
    hw1 = nc.declare_dram_parameter("hw1", [HID, HID], f16, isOutput=False)
    hw2 = nc.declare_dram_parameter("hw2", [HID, VOCAB], f32, isOutput=False)
    lb1 = nc.declare_dram_parameter("lb1", [HID, 1], f32, isOutput=False)
    lb2 = nc.declare_dram_parameter("lb2", [VOCAB, 1], f32, isOutput=False)
    iota = nc.declare_dram_parameter("iota", [VOCAB, 1], f32, isOutput=False)
    ones = nc.declare_dram_parameter("ones", [1, 512], f32, isOutput=False)
    ones16 = nc.declare_dram_parameter("ones16", [1, 512], f16, isOutput=False)
    out = nc.declare_dram_parameter("out", [VOCAB, ncols], f32, isOutput=True)

    with tile.TileContext(nc) as tc:
        from contextlib import ExitStack
        with ExitStack() as ctx:
            cpool = ctx.enter_context(tc.tile_pool(name="consts", bufs=1))
            ohpool = ctx.enter_context(tc.tile_pool(name="oh", bufs=2))
            tkpool = ctx.enter_context(tc.tile_pool(name="tk", bufs=2))
            tbps = ctx.enter_context(
                tc.tile_pool(name="tbps", bufs=1, space="PSUM"))
            gps = ctx.enter_context(
                tc.tile_pool(name="gps", bufs=3, space="PSUM"))
            sps = ctx.enter_context(
                tc.tile_pool(name="sps", bufs=2, space="PSUM"))
            spool = ctx.enter_context(tc.tile_pool(name="sg", bufs=3))
            cc = ctx.enter_context(tc.tile_pool(name="cc", bufs=3))
            r2p = ctx.enter_context(tc.tile_pool(name="r2", bufs=3))
            fcp = ctx.enter_context(tc.tile_pool(name="fcb", bufs=3))
            up = ctx.enter_context(tc.tile_pool(name="ub", bufs=3))
            seqp = ctx.enter_context(tc.tile_pool(name="seq", bufs=2))
            hmps = ctx.enter_context(
                tc.tile_pool(name="hmps", bufs=1, space="PSUM"))
            hmp = ctx.enter_context(tc.tile_pool(name="hm", bufs=2))
            lgps = ctx.enter_context(
                tc.tile_pool(name="lgps", bufs=1, space="PSUM"))
            lgp = ctx.enter_context(tc.tile_pool(name="lg", bufs=3))

            def load_const(ap, shape, tag, dt=f32):
                t = cpool.tile(list(shape), dt, tag=tag)
                nc.sync.dma_start(out=t[:], in_=ap[:])
                return t

            mx_t = load_const(mx, (VOCAB, G4H), "mx", f16)
            w1h_t = load_const(w1h, (HID, G4H), "w1h", f16)
            w2ih_t = load_const(w2ih, (HID, G4H), "w2ih", f16)
            w2hh_t = load_const(w2hh, (HID, G4H), "w2hh", f16)
            b1_t = load_const(b1, (1, G4H), "b1", f16)
            b2_t = load_const(b2, (1, G4H), "b2", f16)
            hw1_t = load_const(hw1, (HID, HID), "hw1", f16)
            hw2_t = load_const(hw2, (HID, VOCAB), "hw2")
            lb1_t = load_const(lb1, (HID, 1), "lb1")
            lb2_t = load_const(lb2, (VOCAB, 1), "lb2")
            iota_t = load_const(iota, (VOCAB, 1), "iota")
            ones_t = load_const(ones, (1, 512), "ones")
            ones16_t = load_const(ones16, (1, 512), "ones16", f16)

            c_prev = None           # AP of previous c_hat tile
            seq_cur = None          # current chunk h-seq tile
            seq_prev_tile = None    # previous chunk h-seq tile
            oh_cur = None

            def head_group(seq_tile, blk0, nblk, t0):
                """MLP head over ĥ2 blocks [blk0, blk0+nblk) of seq_tile;
                t0 = timestep of block blk0; writes out cols t0*BS.."""
                n = nblk * BS
                r = seq_tile[0:HID, :].rearrange(
                    "p (j c) -> p j c", c=2 * BS)
                rhs = r[:, blk0:blk0 + nblk, BS:2 * BS]
                hm_ps = hmps.tile([HID, 512], f32)
                nc.tensor.matmul(hm_ps[:, 0:n], hw1_t[:], rhs,
                                 start=True, stop=True)
                hm = hmp.tile([32, 512], f32)
                nc.scalar.activation(hm[0:HID, 0:n], hm_ps[0:HID, 0:n],
                                     AF.Relu, bias=lb1_t[:, 0:1])
                lg_ps = lgps.tile([VOCAB, 512], f32)
                nc.tensor.matmul(lg_ps[:, 0:n], hw2_t[:], hm[0:HID, 0:n],
                                 start=True, stop=True)
                lg = lgp.tile([VOCAB, 512], f32)
                nc.scalar.activation(lg[:, 0:n], lg_ps[:, 0:n],
                                     AF.Identity, bias=lb2_t[:, 0:1])
                nc.sync.dma_start(out=out[:, t0 * BS:t0 * BS + n],
                                  in_=lg[:, 0:n])

            for s in range(t_steps + 1):
                chunk, j = s // CH, s % CH
                has1 = s < t_steps
                has2 = s >= 1
                if j == 0:
                    seq_prev_tile, seq_cur = seq_cur, seqp.tile(
                        [32, CH * 2 * BS], f16)
                    if has1:
                        cl = min(CH, t_steps - chunk * CH) * BS
                        tk = tkpool.tile([1, CH * BS], f16)
                        nc.sync.dma_start(
                            out=tk[:, 0:cl],
                            in_=tok[:, chunk * CH * BS:chunk * CH * BS + cl])
                        oh_cur = ohpool.tile([VOCAB, CH * BS], f16)
                        nb = cl // 512
                        for blk in range(nb):
                            tb = tbps.tile([VOCAB, 512], f32)
                            nc.tensor.matmul(
                                tb[:], ones16_t[:, 0:VOCAB],
                                tk[:, blk * 512:(blk + 1) * 512],
                                start=True, stop=True)
                            nc.vector.tensor_scalar(
                                out=oh_cur[:, blk * 512:(blk + 1) * 512],
                                in0=tb[:], scalar1=iota_t[:, 0:1],
                                scalar2=None, op0=A.is_equal)

                # previous superstep's state APs
                if s >= 1:
                    pj = (s - 1) % CH
                    ptile = seq_cur if j != 0 else seq_prev_tile
                    h1_prev = ptile[0:HID, pj * 2 * BS:pj * 2 * BS + BS]
                    h2_prev = ptile[0:HID, pj * 2 * BS + BS:(pj + 1) * 2 * BS]

                g_ps = gps.tile([G4H, 2 * BS], f32)
                if has1:
                    # b1 is folded into mx (one-hot columns sum to 1)
                    nc.tensor.matmul(g_ps[:, 0:BS], mx_t[:],
                                     oh_cur[:, j * BS:(j + 1) * BS],
                                     start=True, stop=(s == 0))
                    if s >= 1:
                        nc.tensor.matmul(g_ps[:, 0:BS], w1h_t[:], h1_prev,
                                         start=False, stop=True)
                if has2:
                    nc.tensor.matmul(g_ps[:, BS:2 * BS], b2_t[:],
                                     ones16_t[:, 0:BS], start=True, stop=False)
                    nc.tensor.matmul(g_ps[:, BS:2 * BS], w2ih_t[:], h1_prev,
                                     start=False, stop=(s == 1))
                    if s >= 2:
                        nc.tensor.matmul(g_ps[:, BS:2 * BS], w2hh_t[:],
                                         h2_prev, start=False, stop=True)

                lo = 0 if has1 else BS
                hi = 2 * BS if has2 else BS
                cs = slice(lo, hi)

                # sigmoid(i,f,o) -> PSUM s_ps (i 0:30, f 32:62, o 64:94);
                # tanh(g) -> SBUF st rows 96:126.  Every DVE op below mixes
                # one PSUM and one SBUF operand (exempts the equal-base
                # rule).  fc depends only on the sigmoid instr, so it
                # overlaps the tanh on ACT.
                st = spool.tile([G4H, 2 * BS], f32)
                s_ps = sps.tile([96, 2 * BS], f32)
                nc.scalar.activation(s_ps[0:96, cs], g_ps[0:96, cs],
                                     AF.Sigmoid)
                nc.scalar.activation(st[96:126, cs], g_ps[96:126, cs],
                                     AF.Tanh)

                c_new = cc.tile([32, 2 * BS], f32)
                if s == 0:
                    # c1[0] = s_i * tanh(g); c2[-1] = 0
                    nc.vector.tensor_tensor(
                        out=c_new[0:HID, 0:BS], in0=st[96:126, 0:BS],
                        in1=s_ps[0:HID, 0:BS], op=A.mult)
                    nc.vector.memset(c_new[0:HID, BS:2 * BS], 0.0)
                else:
                    fc = fcp.tile([32, 2 * BS], f32)
                    nc.vector.tensor_tensor(
                        out=fc[0:HID, cs], in0=s_ps[32:62, cs],
                        in1=c_prev[0:HID, cs], op=A.mult)
                    r2 = r2p.tile([32, 2 * BS], f32)
                    nc.vector.tensor_tensor(
                        out=r2[0:HID, cs], in0=st[96:126, cs],
                        in1=s_ps[0:HID, cs], op=A.mult)
                    nc.vector.tensor_tensor(
                        out=c_new[0:HID, cs], in0=fc[0:HID, cs],
                        in1=r2[0:HID, cs], op=A.add)
                c_prev = c_new

                u = up.tile([32, 2 * BS], f32)
                nc.scalar.activation(u[0:HID, cs], c_new[0:HID, cs],
                                     AF.Tanh)
                # h = sigmoid(o) * tanh(c) -> current seq block j (fp16)
                nc.vector.tensor_tensor(
                    out=seq_cur[0:HID, j * 2 * BS + lo:j * 2 * BS + hi],
                    in0=u[0:HID, cs], in1=s_ps[64:94, cs], op=A.mult)

                # head: after block 8g+7 of each chunk is written
                if has2 and (j % 8 == 7):
                    grp = j // 8
                    blk0, nblk = grp * 8, 8
                    t0 = chunk * CH + blk0 - 1
                    if chunk == 0 and grp == 0:
                        blk0, nblk, t0 = 1, 7, 0
                    head_group(seq_cur, blk0, nblk, t0)
                if s == t_steps:
                    # flush: ĥ2[t-1] sits in block j of current chunk
                    head_group(seq_cur, j, 1, s - 1)

    nc.compile()
    return nc


def _get_program():
    global _PROGRAM
    if _PROGRAM is None:
        _PROGRAM = build_program(T)
    return _PROGRAM


def _prep_weights(inp):
    f = lambda k: np.asarray(inp[k], np.float32)
    emb = f("embedding")
    mx = _expand_gate_cols(emb @ f("w_ih1").T)                  # [100,128]
    b1 = _expand_gate_cols((f("b_ih1") + f("b_hh1")))[None, :]  # [1,128]
    mx = mx + b1   # one-hot columns sum to 1 -> adds b1 exactly
    w1h = _expand_gate_cols(f("w_hh1").T)                       # [30,128]
    w2ih = _expand_gate_cols(f("w_ih2").T)
    w2hh = _expand_gate_cols(f("w_hh2").T)
    b2 = _expand_gate_cols((f("b_ih2") + f("b_hh2")))[None, :]
    hw1 = np.ascontiguousarray(f("lw1").T)                      # [30,30]
    hw2 = np.ascontiguousarray(f("lw2").T)                      # [30,100]
    lb1 = f("lb1")[:, None]
    lb2 = f("lb2")[:, None]
    iota = np.arange(VOCAB, dtype=np.float32)[:, None]
    ones = np.ones((1, 512), np.float32)
    h = np.float16
    return dict(mx=mx.astype(h), b1=b1.astype(h), w1h=w1h.astype(h),
                w2ih=w2ih.astype(h), w2hh=w2hh.astype(h), b2=b2.astype(h),
                hw1=hw1.astype(h), hw2=hw2, lb1=lb1, lb2=lb2, iota=iota,
                ones=ones, ones16=np.ones((1, 512), h))


def kernel(**inputs):
    from concourse.bass_utils import run_bass_kernel_spmd

    nc = _get_program()
    batch_x = np.asarray(inputs["batch_x"])
    lens = np.asarray(inputs["batch_x_lens"]).astype(np.int64)
    consts = _prep_weights(inputs)

    in_maps = []
    for m in range(NCORES):
        toks = batch_x[m * BS:(m + 1) * BS].astype(np.float16)  # [BS, T]
        tok_flat = np.ascontiguousarray(toks.T).reshape(1, T * BS)
        in_maps.append({"tok": tok_flat, **consts})

    res = run_bass_kernel_spmd(nc, in_maps, list(range(NCORES)))
    outs = []
    for m in range(NCORES):
        o = res.results[m]["out"]                 # [100, T*BS]
        o = o.reshape(VOCAB, T, BS).transpose(2, 1, 0)  # [BS, T, 100]
        outs.append(o)
    full = np.concatenate(outs, axis=0)           # [B, T, 100]
    # reference zeroes h2 BEFORE the head, so masked positions equal the
    # constant head(0) = relu(lb1) @ lw2.T + lb2
    lb1 = np.asarray(inputs["lb1"], np.float32)
    lw2 = np.asarray(inputs["lw2"], np.float32)
    lb2 = np.asarray(inputs["lb2"], np.float32)
    head0 = np.maximum(lb1, 0.0) @ lw2.T + lb2    # [100]
    mask = np.arange(T)[None, :] < lens[:, None]
    full = np.where(mask[:, :, None], full, head0[None, None, :])
    return np.ascontiguousarray(full.astype(np.float32))


if __name__ == "__main__":
    ref = {}
    exec(open(os.path.join(os.path.dirname(__file__), "reference.py")).read(),
         ref)
    inputs = {k: np.asarray(v) for k, v in ref["setup_inputs"]().items()}
    got = kernel(**inputs)
    want = np.asarray(ref["reference"](**inputs))
    err = np.abs(got - want).max() / max(np.abs(want).max(), 1e-9)
    print("rel err:", err)
